# revision 21
# baseline (speedup 1.0000x reference)
"""DistSAGE 3-layer GraphSAGE forward on 8 TRN2 NeuronCores (Bass/Tile).

Strategy (graph/data parallel, per the DistSAGE recipe):
  - Partition the 512 seed nodes across 8 cores (64 each, LPT-balanced by
    an additive 2-hop cost estimate); build per-core dependency-driven
    blocks on the host (standard DGL block construction, pure index math).
    No inter-core communication; weights replicated.
  - Layer 0 (the memory-bound bulk) streams, per 128-dst tile:
      * the tile's unique source rows as an fp8e4 compact table (one dense
        DMA, pre-interleaved in SBUF layout);
      * host-baked fp8 S' masks (S'[p,d] = sum of 1/deg over edges
        band_row->d).
      The aggregation runs with the MASK as the PE-stationary operand and
      the 256-feature messages as the moving stream (DoubleRow fp8, two
      128-src chunks per instruction) -- one LDWEIGHTS per chunk-pair
      instead of two, since LDWEIGHTS (1 col/cycle) is what binds the PE
      pipe here.  agg lands dst-major; two PE-transpose ops flip it
      feature-major for the tail.
      * the tile's dst rows are PRE-TRANSPOSED on the host (bf16,
        feature-major) so the self-term matmul needs no on-chip transpose.
      Then Y[d,:] = h_dstT.T@W_self + aggT.T@W_neigh + 1s^T@bias in one
      PSUM accumulation, ReLU straight from PSUM, bf16 tile to DRAM.
  - Layers 1/2 read their (runtime-produced) h buffers with the custom
    dma_gather ucode.  The ucode generates descriptors at ~8ns/row on
    GpSimd, so the layer-1 gather is split into source-row RANGES, each
    call's in_ap sliced to the rows it actually reads -- the early-range
    calls overlap layer 0's tail instead of serializing after it.
"""

import heapq

import numpy as np

P = 128
NCORES = 8
NUM_DST = (61952, 5632, 512)
FEAT = 256
OUTW = (256, 256, 19)
SEEDS_PER_CORE = NUM_DST[2] // NCORES  # 64
WINDOW = 32768
NRANGES = (6, 2)  # gather range-split for layers 1, 2


def _bf16():
    import ml_dtypes

    return ml_dtypes.bfloat16


def _f8():
    import ml_dtypes

    return ml_dtypes.float8_e4m3fn


def _q8(a):
    # TRN fp8e4 matches OCP e4m3fn on +-0..240; clip to stay in range.
    return np.clip(a, -240.0, 240.0).astype(_f8())


# ---------------------------------------------------------------------------
# Host-side block construction
# ---------------------------------------------------------------------------


def _balance(ids, deg, n_buckets):
    """LPT bin-packing: reorder ids so consecutive 128-groups have ~equal
    total degree (only full 128-groups are balanced)."""
    if n_buckets <= 1 or len(ids) < n_buckets * P:
        return ids
    order = np.argsort(-deg[ids], kind="stable")
    heap = [(0.0, b, 0) for b in range(n_buckets)]
    heapq.heapify(heap)
    buckets = [[] for _ in range(n_buckets)]
    for i in order:
        load, b, cnt = heapq.heappop(heap)
        buckets[b].append(ids[i])
        cnt += 1
        if cnt < P:
            heapq.heappush(heap, (load + deg[ids[i]], b, cnt))
    return np.concatenate([np.asarray(b, dtype=ids.dtype) for b in buckets])


def _seed_partition(esrc0, edst0, esrc1, edst1, esrc2, edst2, deg0, deg1):
    """LPT-balance seeds across cores by an additive 2-hop cost estimate."""
    h = np.zeros(NUM_DST[1], np.float64)
    np.add.at(h, edst1, deg0[esrc1].astype(np.float64))
    cost = np.zeros(NUM_DST[2], np.float64)
    np.add.at(cost, edst2, h[esrc2] + deg1[esrc2].astype(np.float64))
    order = np.argsort(-cost, kind="stable")
    heap = [(0.0, cc, 0) for cc in range(NCORES)]
    heapq.heapify(heap)
    groups = [[] for _ in range(NCORES)]
    for s in order:
        load, cc, cnt = heapq.heappop(heap)
        groups[cc].append(s)
        cnt += 1
        if cnt < SEEDS_PER_CORE:
            heapq.heappush(heap, (load + cost[s], cc, cnt))
    return [np.array(g, dtype=np.int64) for g in groups]


def _block_for_core(seeds, esrc0, edst0, esrc1, edst1, esrc2, edst2,
                    deg0, deg1, deg2):
    pos2 = np.full(NUM_DST[2], -1, np.int32)
    pos2[seeds] = np.arange(SEEDS_PER_CORE, dtype=np.int32)
    sel2 = pos2[edst2] >= 0
    es2, ed2g = esrc2[sel2], edst2[sel2]
    l1_extra = np.setdiff1d(np.unique(es2), seeds)
    nfull = (len(l1_extra) // P) * P
    if nfull >= P:
        l1_extra = np.concatenate(
            [_balance(l1_extra[:nfull], deg1, nfull // P), l1_extra[nfull:]]
        )
    l1_out = np.concatenate([seeds, l1_extra])
    n1 = len(l1_out)

    pos1 = np.full(NUM_DST[1], -1, np.int32)
    pos1[l1_out] = np.arange(n1, dtype=np.int32)
    sel1 = pos1[edst1] >= 0
    es1, ed1g = esrc1[sel1], edst1[sel1]
    ed1 = pos1[ed1g].astype(np.int64)
    inv1 = (1.0 / np.maximum(deg1[ed1g], 1.0)).astype(np.float32)
    l0_extra = np.setdiff1d(np.unique(es1), l1_out)
    nfull = (len(l0_extra) // P) * P
    if nfull >= P:
        l0_extra = np.concatenate(
            [_balance(l0_extra[:nfull], deg0, nfull // P), l0_extra[nfull:]]
        )
    l0_out = np.concatenate([l1_out, l0_extra])
    n0 = len(l0_out)

    pos0 = np.full(NUM_DST[0], -1, np.int32)
    pos0[l0_out] = np.arange(n0, dtype=np.int32)
    sel0 = pos0[edst0] >= 0
    es0, ed0g = esrc0[sel0], edst0[sel0]
    ed0 = pos0[ed0g].astype(np.int64)
    inv0 = (1.0 / np.maximum(deg0[ed0g], 1.0)).astype(np.float32)

    ed2 = pos2[ed2g].astype(np.int64)
    inv2 = (1.0 / np.maximum(deg2[ed2g], 1.0)).astype(np.float32)
    es2l = pos1[es2].astype(np.int64)
    es1l = pos0[es1].astype(np.int64)

    return dict(
        seeds=seeds, l1_out=l1_out, l0_out=l0_out, n1=n1, n0=n0,
        e0=(es0.astype(np.int64), ed0, inv0),
        e1=(es1l, ed1, inv1),
        e2=(es2l, ed2, inv2),
    )


def _group_edges_by_tile(es, ed, inv, n_tiles):
    """Per dst-tile: dedup sources, build the dense S' payload.
    Returns per-tile (unique_srcs sorted, W [n_u, 128] f32)."""
    tile = ed // P
    order = np.argsort(tile, kind="stable")
    es, ed, inv, tile = es[order], ed[order], inv[order], tile[order]
    starts = np.searchsorted(tile, np.arange(n_tiles))
    ends = np.searchsorted(tile, np.arange(n_tiles) + 1)
    out = []
    for t in range(n_tiles):
        s, e = starts[t], ends[t]
        u, ii = np.unique(es[s:e], return_inverse=True)
        W = np.zeros((len(u), P), np.float32)
        np.add.at(W, (ii, ed[s:e] - t * P), inv[s:e])
        out.append((u, W))
    return out


class GatherPlan:
    """Layers 1/2: slot stream gathered via dma_gather, split into
    source-row RANGES.  Stream order: (range r, tile t) segments; each
    range padded to a chunk (128-slot) boundary so each range is one
    gather call whose in_ap covers only rows < (r+1)*step."""

    def __init__(self, n_tiles, n_ranges, step_rows, seg_counts):
        self.n_tiles = n_tiles
        self.n_ranges = n_ranges
        self.step_rows = step_rows
        self.seg_counts = seg_counts  # [r][t] max-over-core count
        self.seg_off = np.zeros((n_ranges, n_tiles), np.int64)
        self.range_chunk_off = [0]
        slot = 0
        for r in range(n_ranges):
            for t in range(n_tiles):
                self.seg_off[r, t] = slot
                slot += seg_counts[r][t]
            slot = -(-slot // P) * P
            self.range_chunk_off.append(slot // P)
        self.n_chunks = slot // P
        self.n_chunks_pad = self.n_chunks
        # sp columns: one per (tile, chunk) incidence
        self.pairs = []
        self.pair_col = {}
        self.tile_pairs = [[] for _ in range(n_tiles)]
        for r in range(n_ranges):
            for t in range(n_tiles):
                lo = int(self.seg_off[r, t])
                hi = lo + seg_counts[r][t]
                if hi == lo:
                    continue
                for ch in range(lo // P, (hi - 1) // P + 1):
                    col = len(self.pairs)
                    self.pair_col[(t, ch)] = col
                    self.tile_pairs[t].append((col, ch))
                    self.pairs.append((t, ch))
        self.n_sp_cols = len(self.pairs)
        self.gidx = []  # [NCORES][128, n_chunks] int64 table rows
        self.wmat = []  # [NCORES][128, n_sp_cols, 128] f32


def _plan_gather(per_core_tiles, n_tiles, n_ranges, n_rows):
    step = -(-(n_rows // P) // n_ranges) * P
    seg_counts = []
    for r in range(n_ranges):
        row = []
        for t in range(n_tiles):
            cnt = 0
            for c in range(NCORES):
                u, _ = per_core_tiles[c][t]
                cnt = max(cnt, int(np.searchsorted(u, (r + 1) * step)
                                   - np.searchsorted(u, r * step)))
            row.append(cnt)
        seg_counts.append(row)
    # guarantee at least one slot overall so n_chunks >= 1
    if sum(sum(r) for r in seg_counts) == 0:
        seg_counts[0][0] = 1
    return GatherPlan(n_tiles, n_ranges, step, seg_counts)


def _fill_gather(plan, per_core_tiles):
    total = plan.n_chunks * P
    for c in range(NCORES):
        stream = np.zeros(total, np.int64)
        # default pad: row inside range 0
        for r in range(plan.n_ranges):
            a = int(plan.seg_off[r, 0]) if plan.n_tiles else 0
            b = plan.range_chunk_off[r + 1] * P
            stream[a:b] = r * plan.step_rows
        wmat = np.zeros((P, plan.n_sp_cols, P), np.float32)
        for t in range(plan.n_tiles):
            u, W = per_core_tiles[c][t]
            for r in range(plan.n_ranges):
                i0 = int(np.searchsorted(u, r * plan.step_rows))
                i1 = int(np.searchsorted(u, (r + 1) * plan.step_rows))
                lo = int(plan.seg_off[r, t])
                n = i1 - i0
                stream[lo : lo + n] = u[i0:i1]
                # slots [lo+n, lo+seg_counts) keep the range-safe pad row
                for ch in range(lo // P, (lo + plan.seg_counts[r][t] - 1) // P + 1) \
                        if plan.seg_counts[r][t] else []:
                    col = plan.pair_col[(t, ch)]
                    s0 = ch * P
                    a = max(s0, lo)
                    b = min(s0 + P, lo + n)
                    if a < b:
                        wmat[a - s0 : b - s0, col, :] = W[i0 + (a - lo) : i0 + (b - lo)]
        plan.gidx.append(stream.reshape(plan.n_chunks, P).T.copy())
        plan.wmat.append(wmat)


class BandPlan:
    """Layer 0: per-tile fp8 source bands.  Tile t's sources live in
    chunks sp_off[t]..sp_off[t]+K[t]-1 of the xsrc/sp0 tables."""

    def __init__(self, n_tiles, src_counts):
        self.n_tiles = n_tiles
        self.m = src_counts  # real (max-over-core) source count per tile
        self.K = [max(1, -(-m // P)) for m in src_counts]
        self.n_sp_cols = sum(self.K)
        self.sp_off = np.concatenate([[0], np.cumsum(self.K)]).astype(np.int64)
        self.wmat = []  # [NCORES][128, n_sp_cols, 128] f32


def build_host(inputs):
    esrc0 = np.asarray(inputs["esrc0"]).astype(np.int64)
    edst0 = np.asarray(inputs["edst0"]).astype(np.int64)
    esrc1 = np.asarray(inputs["esrc1"]).astype(np.int64)
    edst1 = np.asarray(inputs["edst1"]).astype(np.int64)
    esrc2 = np.asarray(inputs["esrc2"]).astype(np.int64)
    edst2 = np.asarray(inputs["edst2"]).astype(np.int64)
    x = np.asarray(inputs["x"], dtype=np.float32)

    deg0 = np.bincount(edst0, minlength=NUM_DST[0]).astype(np.float32)
    deg1 = np.bincount(edst1, minlength=NUM_DST[1]).astype(np.float32)
    deg2 = np.bincount(edst2, minlength=NUM_DST[2]).astype(np.float32)

    seed_groups = _seed_partition(esrc0, edst0, esrc1, edst1, esrc2, edst2,
                                  deg0, deg1)
    blocks = [
        _block_for_core(seed_groups[c], esrc0, edst0, esrc1, edst1, esrc2,
                        edst2, deg0, deg1, deg2)
        for c in range(NCORES)
    ]

    n0_pad = max(-(-b["n0"] // P) for b in blocks) * P
    n1_pad = max(-(-b["n1"] // P) for b in blocks) * P
    T0, T1, T2 = n0_pad // P, n1_pad // P, 1

    tiles0 = [_group_edges_by_tile(*b["e0"], T0) for b in blocks]
    tiles1 = [_group_edges_by_tile(*b["e1"], T1) for b in blocks]
    tiles2 = [_group_edges_by_tile(*b["e2"], T2) for b in blocks]

    # ---- layer 0: band plan + fp8 source tables + bf16 dstT table ----
    plan0 = BandPlan(
        T0,
        [max(len(tiles0[c][t][0]) for c in range(NCORES)) for t in range(T0)],
    )
    l0_padded = []
    for b in blocks:
        v = np.zeros(T0 * P, np.int64)
        v[: b["n0"]] = b["l0_out"]
        v[b["n0"] :] = b["l0_out"][0]
        l0_padded.append(v)

    bf16 = _bf16()
    f8 = _f8()
    x16 = x.astype(bf16)
    x8 = _q8(x)
    xsrc8s, xdstTs = [], []
    for c in range(NCORES):
        xr = np.zeros((P, plan0.n_sp_cols, FEAT), f8)
        wmat = np.zeros((P, plan0.n_sp_cols, P), np.float32)
        for t in range(T0):
            u, W = tiles0[c][t]
            rows = x8[u]
            so = int(plan0.sp_off[t])
            for k in range(plan0.K[t]):
                a, b = k * P, min((k + 1) * P, len(u))
                if a < b:
                    xr[: b - a, so + k, :] = rows[a:b]
                    wmat[: b - a, so + k, :] = W[a:b]
        xsrc8s.append(np.ascontiguousarray(xr.reshape(P, plan0.n_sp_cols * FEAT)))
        plan0.wmat.append(wmat)
        # dstT: tile t at [:, t*FEAT:(t+1)*FEAT], two feature-half blocks of
        # [128 feat, 128 dst] each  (xdstT[p, t*256 + h*128 + d] = x[dst, h*128+p])
        dst_rows = x16[l0_padded[c]]  # [T0*P, FEAT]
        dt = dst_rows.reshape(T0, P, 2, P).transpose(3, 0, 2, 1)  # [p,T0,h,d]
        xdstTs.append(np.ascontiguousarray(dt.reshape(P, T0 * FEAT)))

    # ---- layers 1/2: gather plans ----
    plan1 = _plan_gather(tiles1, T1, NRANGES[0], n0_pad)
    plan2 = _plan_gather(tiles2, T2, NRANGES[1], n1_pad)
    _fill_gather(plan1, tiles1)
    _fill_gather(plan2, tiles2)
    assert n0_pad <= WINDOW and n1_pad <= WINDOW

    return dict(
        plan0=plan0,
        plans=(plan1, plan2),
        T=(T0, T1, T2),
        n0_pad=n0_pad,
        n1_pad=n1_pad,
        xsrc8s=xsrc8s,
        xdstTs=xdstTs,
        blocks=blocks,
        weights=tuple(
            (
                np.asarray(inputs[f"W_self{l}"], np.float32),
                np.asarray(inputs[f"W_neigh{l}"], np.float32),
                np.asarray(inputs[f"b{l}"], np.float32),
            )
            for l in range(3)
        ),
    )


# ---------------------------------------------------------------------------
# Device kernel
# ---------------------------------------------------------------------------


def _wrap_idx16(plan, c):
    """int16 idx table: flat stream wrapped into 16 partitions, replicated
    to 128 (the gather ucode reads 16-partition-wrapped indices)."""
    flat = plan.gidx[c].T.reshape(-1)  # stream order
    w = flat.reshape(len(flat) // 16, 16).T.astype(np.int16)
    out = np.zeros((P, w.shape[1]), np.int16)
    out[:16] = w
    for rep in range(1, 8):
        out[rep * 16 : (rep + 1) * 16] = out[:16]
    return out


def run_device(meta, trace=False):
    import concourse.bacc as bacc
    import concourse.tile as tile
    import concourse.mybir as mybir
    from concourse.bass_utils import run_bass_kernel_spmd

    plan0 = meta["plan0"]
    plans = meta["plans"]
    f32 = mybir.dt.float32
    b16 = mybir.dt.bfloat16
    f8e4 = mybir.dt.float8e4
    DR = mybir.MatmulPerfMode.DoubleRow

    nc = bacc.Bacc("TRN2", target_bir_lowering=False, debug=False, num_devices=NCORES)

    xsrc8 = nc.dram_tensor("xsrc8", [P, plan0.n_sp_cols * FEAT], f8e4,
                           kind="ExternalInput")
    xdstT = nc.dram_tensor("xdstT", [P, plan0.n_tiles * FEAT], b16,
                           kind="ExternalInput")
    sp0_d = nc.dram_tensor("sp0", [P, plan0.n_sp_cols * P], f8e4,
                           kind="ExternalInput")
    ident_d = nc.dram_tensor("ident", [P, P], b16, kind="ExternalInput")
    ones_d = nc.dram_tensor("ones", [1, P], b16, kind="ExternalInput")
    h1buf = nc.dram_tensor("h1buf", [meta["n0_pad"], FEAT], b16)
    h2buf = nc.dram_tensor("h2buf", [meta["n1_pad"], FEAT], b16)
    out_d = nc.dram_tensor("out", [SEEDS_PER_CORE, OUTW[2]], f32, kind="ExternalOutput")

    idx_d, sp_d = [], []
    for li, plan in enumerate(plans):
        idx_d.append(
            nc.dram_tensor(f"gidx{li + 1}", [P, plan.n_chunks * P // 16],
                           mybir.dt.int16, kind="ExternalInput")
        )
        sp_d.append(
            nc.dram_tensor(f"sp{li + 1}", [P, plan.n_sp_cols * P], b16,
                           kind="ExternalInput")
        )
    w_d = []
    for l in range(3):
        w_d.append(
            (
                nc.dram_tensor(f"ws{l}", [FEAT, OUTW[l]], b16, kind="ExternalInput"),
                nc.dram_tensor(f"wn{l}", [FEAT, OUTW[l]], b16, kind="ExternalInput"),
                nc.dram_tensor(f"bias{l}", [P, OUTW[l]], b16, kind="ExternalInput"),
            )
        )

    Kmax = max(plan0.K)

    with tile.TileContext(nc) as tc:
        with (
            tc.tile_pool(name="const", bufs=1) as cpool,
            tc.tile_pool(name="msgs", bufs=6) as mpool,
            tc.tile_pool(name="sel", bufs=6) as spool,
            tc.tile_pool(name="gat", bufs=1) as gpool,
            tc.tile_pool(name="acc", bufs=4) as apool,
            tc.tile_pool(name="outp", bufs=3) as opool,
            tc.tile_pool(name="pagg", bufs=2, space="PSUM") as pa,
            tc.tile_pool(name="py", bufs=2, space="PSUM") as pypool,
        ):
            ident_t = cpool.tile([P, P], b16, tag="ident")
            nc.sync.dma_start(out=ident_t[:], in_=ident_d[:])
            ws_ts, wn_ts, bias_ts = [], [], []
            for l in range(3):
                outw = OUTW[l]
                wst, wnt = [], []
                for k in range(2):
                    w = cpool.tile([P, outw], b16, tag=f"ws{l}_{k}")
                    nc.sync.dma_start(out=w[:], in_=w_d[l][0][k * P : (k + 1) * P, :])
                    wst.append(w)
                    w = cpool.tile([P, outw], b16, tag=f"wn{l}_{k}")
                    nc.sync.dma_start(out=w[:], in_=w_d[l][1][k * P : (k + 1) * P, :])
                    wnt.append(w)
                ws_ts.append(wst)
                wn_ts.append(wnt)
                # bias broadcast across partitions: PSUM gets pre-filled with
                # it by the DVE, replacing a per-tile PE bias matmul
                bias_t = cpool.tile([P, outw], b16, tag=f"bias{l}")
                nc.sync.dma_start(out=bias_t[:], in_=w_d[l][2][:])
                bias_ts.append(bias_t)
            idx_ts = []
            for li, plan in enumerate(plans):
                idx_t = cpool.tile(
                    list(idx_d[li].shape), mybir.dt.int16, tag=f"idx{li + 1}"
                )
                nc.sync.dma_start(out=idx_t[:], in_=idx_d[li][:])
                idx_ts.append(idx_t)

            def agg_to_acT(pag):
                """agg [128d, 256f] psum -> bf16 -> PE-transpose -> acT
                [128f-half x2, 128d] sbuf (feature-major for the tail)."""
                ag = apool.tile([P, FEAT], b16, tag="ag")
                nc.vector.tensor_copy(out=ag[:], in_=pag[:])
                ptr = pa.tile([P, FEAT], b16, tag="ptr")
                for h in range(2):
                    nc.tensor.matmul(ptr[:, h * P : (h + 1) * P],
                                     lhsT=ag[:, h * P : (h + 1) * P],
                                     rhs=ident_t[:], is_transpose=True)
                acT = apool.tile([P, FEAT], b16, tag="acT")
                nc.vector.tensor_copy(out=acT[:], in_=ptr[:])
                return acT

            # (L1/L2 psum tags alias L0's pcA slot size: [P, 256] f32)

            biases_nonzero = [
                bool(np.any(meta["weights"][l][2] != 0.0)) for l in range(3)
            ]

            def tile_tail(l, t, a0, a1, hd0, hd1, dest):
                """Y matmuls + activation + store for one dst tile.
                a0/a1: aggT feature halves [128f, 128d] (bf16);
                hd0/hd1: h_dstT feature halves [128f, 128d] (bf16).
                A nonzero bias is added by the DVE on the way out."""
                outw = OUTW[l]
                y = pypool.tile([P, outw], f32, tag="y")
                nc.tensor.matmul(y[:], lhsT=a0, rhs=wn_ts[l][0][:],
                                 start=True, stop=False)
                nc.tensor.matmul(y[:], lhsT=a1, rhs=wn_ts[l][1][:],
                                 start=False, stop=False)
                nc.tensor.matmul(y[:], lhsT=hd0, rhs=ws_ts[l][0][:],
                                 start=False, stop=False)
                nc.tensor.matmul(y[:], lhsT=hd1, rhs=ws_ts[l][1][:],
                                 start=False, stop=True)
                yv = y[:]
                if biases_nonzero[l]:
                    yb = apool.tile([P, outw], f32, tag="yb")
                    nc.vector.tensor_tensor(
                        out=yb[:], in0=y[:], in1=bias_ts[l][:],
                        op=mybir.AluOpType.add,
                    )
                    yv = yb[:]
                if l < 2:
                    o2 = opool.tile([P, outw], b16, tag="o2")
                    nc.vector.tensor_scalar_max(out=o2[:], in0=yv, scalar1=0.0)
                    nc.sync.dma_start(out=dest[t * P : (t + 1) * P, :], in_=o2[:])
                else:
                    o = opool.tile([P, outw], f32, tag="o")
                    nc.vector.tensor_copy(out=o[:], in_=yv)
                    nc.sync.dma_start(out=dest[:], in_=o[0:SEEDS_PER_CORE, :])

            # ================= layer 0: fp8 bands, DoubleRow agg =============
            pend = None
            for t in range(plan0.n_tiles):
                K = plan0.K[t]
                so = int(plan0.sp_off[t])
                bt = mpool.tile([P, Kmax * FEAT], f8e4, tag="band")
                nc.scalar.dma_start(
                    out=bt[:, : K * FEAT],
                    in_=xsrc8[:, so * FEAT : (so + K) * FEAT],
                )
                spt = spool.tile([P, Kmax * P], f8e4, tag="spb")
                nc.scalar.dma_start(
                    out=spt[:, : K * P], in_=sp0_d[:, so * P : (so + K) * P]
                )
                hdt = apool.tile([P, FEAT], b16, tag="hdt")
                nc.sync.dma_start(
                    out=hdt[:], in_=xdstT[:, t * FEAT : (t + 1) * FEAT]
                )
                # the two feature-half accumulation groups run SEQUENTIALLY:
                # start=True lazily zero-arms the whole 2KB PSUM region, so
                # interleaving groups in one bank corrupts the sibling's
                # partial sums
                pcA = pa.tile([P, 2 * P], f32, tag="pcA")
                npair = K // 2
                odd = K % 2
                for h in range(2):
                    dst = pcA[:, h * P : (h + 1) * P]
                    for kp in range(npair):
                        st, sp = (kp == 0), (kp == npair - 1 and not odd)
                        bv = bt[:, 2 * kp * FEAT : (2 * kp + 2) * FEAT].rearrange(
                            "p (k f) -> p k f", k=2
                        )
                        sv = spt[:, 2 * kp * P : (2 * kp + 2) * P].rearrange(
                            "p (k d) -> p k d", k=2
                        )
                        nc.tensor.matmul(dst, lhsT=bv[:, :, h * P : (h + 1) * P],
                                         rhs=sv[:], start=st, stop=sp,
                                         perf_mode=DR)
                    if odd:
                        k = K - 1
                        base = k * FEAT + h * P
                        nc.tensor.matmul(dst, lhsT=bt[:, base : base + P],
                                         rhs=spt[:, k * P : (k + 1) * P],
                                         start=(npair == 0), stop=True)
                ac = apool.tile([P, 2 * P], b16, tag="ac")
                nc.vector.tensor_copy(out=ac[:], in_=pcA[:])
                # software-pipeline: emit the PREVIOUS tile's tail now, so
                # its DVE psum->sbuf copy overlapped this tile's agg matmuls
                # and the tensor queue never stalls on the copy
                if pend is not None:
                    tile_tail(0, *pend)
                pend = (t, ac[:, 0:P], ac[:, P : 2 * P],
                        hdt[:, 0:P], hdt[:, P : 2 * P], h1buf)

            if pend is not None:
                tile_tail(0, *pend)

            # ============ layers 1/2: range-split gather + bf16 agg ==========
            tables = [h1buf, h2buf]
            dests = [h2buf, out_d]
            for li, plan in enumerate(plans):
                l = li + 1
                table, dest = tables[li], dests[li]
                idx_t = idx_ts[li]
                nch = plan.n_chunks
                nrows = table.shape[0]

                mt = gpool.tile([P, nch * FEAT], b16, tag=f"msgs{l}")
                for r in range(plan.n_ranges):
                    a = plan.range_chunk_off[r]
                    b2 = plan.range_chunk_off[r + 1]
                    if b2 == a:
                        continue
                    hi = min((r + 1) * plan.step_rows, nrows)
                    nc.gpsimd.dma_gather(
                        out_ap=mt[:, a * FEAT : b2 * FEAT].rearrange(
                            "p (g d) -> p g d", g=b2 - a
                        ),
                        in_ap=table[0:hi, :],
                        idxs_ap=idx_t[:, a * P // 16 : b2 * P // 16],
                        num_idxs=(b2 - a) * P,
                        num_idxs_reg=(b2 - a) * P,
                        elem_size=FEAT,
                        single_packet=False,
                    )

                SPG = 16
                n_slabs = -(-plan.n_sp_cols // SPG)
                sp_tiles = []
                for k in range(n_slabs):
                    c0 = k * SPG * P
                    c1 = min((k + 1) * SPG * P, plan.n_sp_cols * P)
                    st = spool.tile([P, SPG * P], b16, tag=f"sp{l}_{k}", bufs=1)
                    nc.scalar.dma_start(out=st[:, : c1 - c0], in_=sp_d[li][:, c0:c1])
                    sp_tiles.append(st)

                def sp_slice(col):
                    k, j = divmod(col, SPG)
                    return sp_tiles[k][:, j * P : (j + 1) * P]

                pend = None
                for t in range(plan.n_tiles):
                    hd = opool.tile([P, FEAT], b16, tag="hd")
                    nc.scalar.dma_start(out=hd[:], in_=table[t * P : (t + 1) * P, :])
                    pcd = pa.tile([P, 2 * P], f32, tag="pcd")
                    nc.tensor.matmul(pcd[:, 0:P], lhsT=hd[:, 0:P],
                                     rhs=ident_t[:], start=True, stop=True)
                    nc.tensor.matmul(pcd[:, P : 2 * P], lhsT=hd[:, P : 2 * P],
                                     rhs=ident_t[:], start=True, stop=True)
                    pag = pa.tile([P, FEAT], f32, tag="pcA")
                    pairs = plan.tile_pairs[t]
                    for i, (sp_col, ch) in enumerate(pairs):
                        st, sp = (i == 0), (i == len(pairs) - 1)
                        nc.tensor.matmul(pag[:], lhsT=sp_slice(sp_col),
                                         rhs=mt[:, ch * FEAT : (ch + 1) * FEAT],
                                         start=st, stop=sp)
                    hdT = apool.tile([P, 2 * P], b16, tag="hdT")
                    nc.vector.tensor_copy(out=hdT[:], in_=pcd[:])
                    if pend is not None:
                        acT = agg_to_acT(pend[0])
                        tile_tail(l, pend[1], acT[:, 0:P], acT[:, P : 2 * P],
                                  pend[2][:, 0:P], pend[2][:, P : 2 * P], dest)
                    pend = (pag, t, hdT)
                if pend is not None:
                    acT = agg_to_acT(pend[0])
                    tile_tail(l, pend[1], acT[:, 0:P], acT[:, P : 2 * P],
                              pend[2][:, 0:P], pend[2][:, P : 2 * P], dest)

    nc.compile()

    in_maps = []
    bf16 = _bf16()
    eye16 = np.eye(P, dtype=bf16)
    for c in range(NCORES):
        m = dict(
            xsrc8=meta["xsrc8s"][c],
            xdstT=meta["xdstTs"][c],
            sp0=np.ascontiguousarray(
                _q8(plan0.wmat[c]).reshape(P, plan0.n_sp_cols * P)
            ),
            ident=eye16,
            ones=np.ones((1, P), dtype=bf16),
        )
        for li, plan in enumerate(plans):
            m[f"gidx{li + 1}"] = _wrap_idx16(plan, c)
            m[f"sp{li + 1}"] = np.ascontiguousarray(
                plan.wmat[c].astype(bf16).reshape(P, plan.n_sp_cols * P)
            )
        for l in range(3):
            ws, wn, b = meta["weights"][l]
            m[f"ws{l}"] = np.ascontiguousarray(ws.astype(bf16))
            m[f"wn{l}"] = np.ascontiguousarray(wn.astype(bf16))
            m[f"bias{l}"] = np.ascontiguousarray(
                np.tile(b[None, :], (P, 1)).astype(bf16)
            )
        in_maps.append(m)

    res = run_bass_kernel_spmd(
        nc, in_maps, core_ids=list(range(NCORES)), trace=trace
    )
    return [res.results[c]["out"] for c in range(NCORES)], res


def assemble(meta, outs):
    full = np.zeros((NUM_DST[2], OUTW[2]), np.float32)
    for c in range(NCORES):
        full[meta["blocks"][c]["seeds"]] = outs[c]
    return full


def kernel(**inputs) -> np.ndarray:
    meta = build_host(inputs)
    outs, _ = run_device(meta)
    return assemble(meta, outs)


# revision 22
# speedup vs baseline: 1.0308x; 1.0308x over previous
"""DistSAGE 3-layer GraphSAGE forward on 8 TRN2 NeuronCores (Bass/Tile).

Strategy (graph/data parallel, per the DistSAGE recipe):
  - Partition the 512 seed nodes across 8 cores (64 each, LPT-balanced by
    an additive 2-hop cost estimate); build per-core dependency-driven
    blocks on the host (standard DGL block construction, pure index math).
    No inter-core communication; weights replicated.
  - Layer 0 (the memory-bound bulk) streams, per 128-dst tile:
      * the tile's unique source rows as an fp8e4 compact table (one dense
        DMA, pre-interleaved in SBUF layout);
      * host-baked fp8 S' masks (S'[p,d] = sum of 1/deg over edges
        band_row->d).
      The aggregation runs with the MASK as the PE-stationary operand and
      the 256-feature messages as the moving stream (DoubleRow fp8, two
      128-src chunks per instruction) -- one LDWEIGHTS per chunk-pair
      instead of two, since LDWEIGHTS (1 col/cycle) is what binds the PE
      pipe here.  agg lands dst-major; two PE-transpose ops flip it
      feature-major for the tail.
      * the tile's dst rows are PRE-TRANSPOSED on the host (bf16,
        feature-major) so the self-term matmul needs no on-chip transpose.
      Then Y[d,:] = h_dstT.T@W_self + aggT.T@W_neigh + 1s^T@bias in one
      PSUM accumulation, ReLU straight from PSUM, bf16 tile to DRAM.
  - Layers 1/2 read their (runtime-produced) h buffers with the custom
    dma_gather ucode.  The ucode generates descriptors at ~8ns/row on
    GpSimd, so the layer-1 gather is split into source-row RANGES, each
    call's in_ap sliced to the rows it actually reads -- the early-range
    calls overlap layer 0's tail instead of serializing after it.
"""

import heapq

import numpy as np

P = 128
NCORES = 8
NUM_DST = (61952, 5632, 512)
FEAT = 256
OUTW = (256, 256, 19)
SEEDS_PER_CORE = NUM_DST[2] // NCORES  # 64
WINDOW = 32768
NRANGES = (6, 2)  # gather range-split for layers 1, 2


def _bf16():
    import ml_dtypes

    return ml_dtypes.bfloat16


def _f8():
    import ml_dtypes

    return ml_dtypes.float8_e4m3fn


def _q8(a):
    # TRN fp8e4 matches OCP e4m3fn on +-0..240; clip to stay in range.
    return np.clip(a, -240.0, 240.0).astype(_f8())


# ---------------------------------------------------------------------------
# Host-side block construction
# ---------------------------------------------------------------------------


def _balance(ids, deg, n_buckets):
    """LPT bin-packing: reorder ids so consecutive 128-groups have ~equal
    total degree (only full 128-groups are balanced)."""
    if n_buckets <= 1 or len(ids) < n_buckets * P:
        return ids
    order = np.argsort(-deg[ids], kind="stable")
    heap = [(0.0, b, 0) for b in range(n_buckets)]
    heapq.heapify(heap)
    buckets = [[] for _ in range(n_buckets)]
    for i in order:
        load, b, cnt = heapq.heappop(heap)
        buckets[b].append(ids[i])
        cnt += 1
        if cnt < P:
            heapq.heappush(heap, (load + deg[ids[i]], b, cnt))
    return np.concatenate([np.asarray(b, dtype=ids.dtype) for b in buckets])


def _seed_partition(esrc0, edst0, esrc1, edst1, esrc2, edst2, deg0, deg1):
    """LPT-balance seeds across cores by an additive 2-hop cost estimate."""
    h = np.zeros(NUM_DST[1], np.float64)
    np.add.at(h, edst1, deg0[esrc1].astype(np.float64))
    cost = np.zeros(NUM_DST[2], np.float64)
    np.add.at(cost, edst2, h[esrc2] + deg1[esrc2].astype(np.float64))
    order = np.argsort(-cost, kind="stable")
    heap = [(0.0, cc, 0) for cc in range(NCORES)]
    heapq.heapify(heap)
    groups = [[] for _ in range(NCORES)]
    for s in order:
        load, cc, cnt = heapq.heappop(heap)
        groups[cc].append(s)
        cnt += 1
        if cnt < SEEDS_PER_CORE:
            heapq.heappush(heap, (load + cost[s], cc, cnt))
    return [np.array(g, dtype=np.int64) for g in groups]


def _block_for_core(seeds, esrc0, edst0, esrc1, edst1, esrc2, edst2,
                    deg0, deg1, deg2):
    pos2 = np.full(NUM_DST[2], -1, np.int32)
    pos2[seeds] = np.arange(SEEDS_PER_CORE, dtype=np.int32)
    sel2 = pos2[edst2] >= 0
    es2, ed2g = esrc2[sel2], edst2[sel2]
    l1_extra = np.setdiff1d(np.unique(es2), seeds)
    nfull = (len(l1_extra) // P) * P
    if nfull >= P:
        l1_extra = np.concatenate(
            [_balance(l1_extra[:nfull], deg1, nfull // P), l1_extra[nfull:]]
        )
    l1_out = np.concatenate([seeds, l1_extra])
    n1 = len(l1_out)

    pos1 = np.full(NUM_DST[1], -1, np.int32)
    pos1[l1_out] = np.arange(n1, dtype=np.int32)
    sel1 = pos1[edst1] >= 0
    es1, ed1g = esrc1[sel1], edst1[sel1]
    ed1 = pos1[ed1g].astype(np.int64)
    inv1 = (1.0 / np.maximum(deg1[ed1g], 1.0)).astype(np.float32)
    l0_extra = np.setdiff1d(np.unique(es1), l1_out)
    nfull = (len(l0_extra) // P) * P
    if nfull >= P:
        l0_extra = np.concatenate(
            [_balance(l0_extra[:nfull], deg0, nfull // P), l0_extra[nfull:]]
        )
    l0_out = np.concatenate([l1_out, l0_extra])
    n0 = len(l0_out)

    pos0 = np.full(NUM_DST[0], -1, np.int32)
    pos0[l0_out] = np.arange(n0, dtype=np.int32)
    sel0 = pos0[edst0] >= 0
    es0, ed0g = esrc0[sel0], edst0[sel0]
    ed0 = pos0[ed0g].astype(np.int64)
    inv0 = (1.0 / np.maximum(deg0[ed0g], 1.0)).astype(np.float32)

    ed2 = pos2[ed2g].astype(np.int64)
    inv2 = (1.0 / np.maximum(deg2[ed2g], 1.0)).astype(np.float32)
    es2l = pos1[es2].astype(np.int64)
    es1l = pos0[es1].astype(np.int64)

    return dict(
        seeds=seeds, l1_out=l1_out, l0_out=l0_out, n1=n1, n0=n0,
        e0=(es0.astype(np.int64), ed0, inv0),
        e1=(es1l, ed1, inv1),
        e2=(es2l, ed2, inv2),
    )


def _group_edges_by_tile(es, ed, inv, n_tiles):
    """Per dst-tile: dedup sources, build the dense S' payload.
    Returns per-tile (unique_srcs sorted, W [n_u, 128] f32)."""
    tile = ed // P
    order = np.argsort(tile, kind="stable")
    es, ed, inv, tile = es[order], ed[order], inv[order], tile[order]
    starts = np.searchsorted(tile, np.arange(n_tiles))
    ends = np.searchsorted(tile, np.arange(n_tiles) + 1)
    out = []
    for t in range(n_tiles):
        s, e = starts[t], ends[t]
        u, ii = np.unique(es[s:e], return_inverse=True)
        W = np.zeros((len(u), P), np.float32)
        np.add.at(W, (ii, ed[s:e] - t * P), inv[s:e])
        out.append((u, W))
    return out


class GatherPlan:
    """Layers 1/2: slot stream gathered via dma_gather, split into
    source-row RANGES.  Stream order: (range r, tile t) segments; each
    range padded to a chunk (128-slot) boundary so each range is one
    gather call whose in_ap covers only rows < (r+1)*step."""

    def __init__(self, n_tiles, n_ranges, step_rows, seg_counts):
        self.n_tiles = n_tiles
        self.n_ranges = n_ranges
        self.step_rows = step_rows
        self.seg_counts = seg_counts  # [r][t] max-over-core count
        self.seg_off = np.zeros((n_ranges, n_tiles), np.int64)
        self.range_chunk_off = [0]
        slot = 0
        for r in range(n_ranges):
            for t in range(n_tiles):
                self.seg_off[r, t] = slot
                slot += seg_counts[r][t]
            slot = -(-slot // P) * P
            self.range_chunk_off.append(slot // P)
        self.n_chunks = slot // P
        self.n_chunks_pad = self.n_chunks
        # sp columns: one per (tile, chunk) incidence
        self.pairs = []
        self.pair_col = {}
        self.tile_pairs = [[] for _ in range(n_tiles)]
        for r in range(n_ranges):
            for t in range(n_tiles):
                lo = int(self.seg_off[r, t])
                hi = lo + seg_counts[r][t]
                if hi == lo:
                    continue
                for ch in range(lo // P, (hi - 1) // P + 1):
                    col = len(self.pairs)
                    self.pair_col[(t, ch)] = col
                    self.tile_pairs[t].append((col, ch))
                    self.pairs.append((t, ch))
        self.n_sp_cols = len(self.pairs)
        self.gidx = []  # [NCORES][128, n_chunks] int64 table rows
        self.wmat = []  # [NCORES][128, n_sp_cols, 128] f32


def _plan_gather(per_core_tiles, n_tiles, n_ranges, n_rows):
    step = -(-(n_rows // P) // n_ranges) * P
    seg_counts = []
    for r in range(n_ranges):
        row = []
        for t in range(n_tiles):
            cnt = 0
            for c in range(NCORES):
                u, _ = per_core_tiles[c][t]
                cnt = max(cnt, int(np.searchsorted(u, (r + 1) * step)
                                   - np.searchsorted(u, r * step)))
            row.append(cnt)
        seg_counts.append(row)
    # guarantee at least one slot overall so n_chunks >= 1
    if sum(sum(r) for r in seg_counts) == 0:
        seg_counts[0][0] = 1
    return GatherPlan(n_tiles, n_ranges, step, seg_counts)


def _fill_gather(plan, per_core_tiles):
    total = plan.n_chunks * P
    for c in range(NCORES):
        stream = np.zeros(total, np.int64)
        # default pad: row inside range 0
        for r in range(plan.n_ranges):
            a = int(plan.seg_off[r, 0]) if plan.n_tiles else 0
            b = plan.range_chunk_off[r + 1] * P
            stream[a:b] = r * plan.step_rows
        wmat = np.zeros((P, plan.n_sp_cols, P), np.float32)
        for t in range(plan.n_tiles):
            u, W = per_core_tiles[c][t]
            for r in range(plan.n_ranges):
                i0 = int(np.searchsorted(u, r * plan.step_rows))
                i1 = int(np.searchsorted(u, (r + 1) * plan.step_rows))
                lo = int(plan.seg_off[r, t])
                n = i1 - i0
                stream[lo : lo + n] = u[i0:i1]
                # slots [lo+n, lo+seg_counts) keep the range-safe pad row
                for ch in range(lo // P, (lo + plan.seg_counts[r][t] - 1) // P + 1) \
                        if plan.seg_counts[r][t] else []:
                    col = plan.pair_col[(t, ch)]
                    s0 = ch * P
                    a = max(s0, lo)
                    b = min(s0 + P, lo + n)
                    if a < b:
                        wmat[a - s0 : b - s0, col, :] = W[i0 + (a - lo) : i0 + (b - lo)]
        plan.gidx.append(stream.reshape(plan.n_chunks, P).T.copy())
        plan.wmat.append(wmat)


class BandPlan:
    """Layer 0: per-tile fp8 source bands.  Tile t's sources live in
    chunks sp_off[t]..sp_off[t]+K[t]-1 of the xsrc/sp0 tables."""

    def __init__(self, n_tiles, src_counts):
        self.n_tiles = n_tiles
        self.m = src_counts  # real (max-over-core) source count per tile
        self.K = [max(1, -(-m // P)) for m in src_counts]
        self.n_sp_cols = sum(self.K)
        self.sp_off = np.concatenate([[0], np.cumsum(self.K)]).astype(np.int64)
        self.wmat = []  # [NCORES][128, n_sp_cols, 128] f32


def build_host(inputs):
    esrc0 = np.asarray(inputs["esrc0"]).astype(np.int64)
    edst0 = np.asarray(inputs["edst0"]).astype(np.int64)
    esrc1 = np.asarray(inputs["esrc1"]).astype(np.int64)
    edst1 = np.asarray(inputs["edst1"]).astype(np.int64)
    esrc2 = np.asarray(inputs["esrc2"]).astype(np.int64)
    edst2 = np.asarray(inputs["edst2"]).astype(np.int64)
    x = np.asarray(inputs["x"], dtype=np.float32)

    deg0 = np.bincount(edst0, minlength=NUM_DST[0]).astype(np.float32)
    deg1 = np.bincount(edst1, minlength=NUM_DST[1]).astype(np.float32)
    deg2 = np.bincount(edst2, minlength=NUM_DST[2]).astype(np.float32)

    seed_groups = _seed_partition(esrc0, edst0, esrc1, edst1, esrc2, edst2,
                                  deg0, deg1)
    blocks = [
        _block_for_core(seed_groups[c], esrc0, edst0, esrc1, edst1, esrc2,
                        edst2, deg0, deg1, deg2)
        for c in range(NCORES)
    ]

    n0_pad = max(-(-b["n0"] // P) for b in blocks) * P
    n1_pad = max(-(-b["n1"] // P) for b in blocks) * P
    T0, T1, T2 = n0_pad // P, n1_pad // P, 1

    tiles0 = [_group_edges_by_tile(*b["e0"], T0) for b in blocks]
    tiles1 = [_group_edges_by_tile(*b["e1"], T1) for b in blocks]
    tiles2 = [_group_edges_by_tile(*b["e2"], T2) for b in blocks]

    # ---- layer 0: band plan + fp8 source tables + bf16 dstT table ----
    plan0 = BandPlan(
        T0,
        [max(len(tiles0[c][t][0]) for c in range(NCORES)) for t in range(T0)],
    )
    l0_padded = []
    for b in blocks:
        v = np.zeros(T0 * P, np.int64)
        v[: b["n0"]] = b["l0_out"]
        v[b["n0"] :] = b["l0_out"][0]
        l0_padded.append(v)

    bf16 = _bf16()
    f8 = _f8()
    x16 = x.astype(bf16)
    x8 = _q8(x)
    xsrc8s, xdstTs = [], []
    for c in range(NCORES):
        xr = np.zeros((P, plan0.n_sp_cols, FEAT), f8)
        wmat = np.zeros((P, plan0.n_sp_cols, P), np.float32)
        for t in range(T0):
            u, W = tiles0[c][t]
            rows = x8[u]
            so = int(plan0.sp_off[t])
            for k in range(plan0.K[t]):
                a, b = k * P, min((k + 1) * P, len(u))
                if a < b:
                    xr[: b - a, so + k, :] = rows[a:b]
                    wmat[: b - a, so + k, :] = W[a:b]
        xsrc8s.append(np.ascontiguousarray(xr.reshape(P, plan0.n_sp_cols * FEAT)))
        plan0.wmat.append(wmat)
        # dstT: tile t at [:, t*FEAT:(t+1)*FEAT], two feature-half blocks of
        # [128 feat, 128 dst] each  (xdstT[p, t*256 + h*128 + d] = x[dst, h*128+p])
        dst_rows = x16[l0_padded[c]]  # [T0*P, FEAT]
        dt = dst_rows.reshape(T0, P, 2, P).transpose(3, 0, 2, 1)  # [p,T0,h,d]
        xdstTs.append(np.ascontiguousarray(dt.reshape(P, T0 * FEAT)))

    # ---- layers 1/2: gather plans ----
    plan1 = _plan_gather(tiles1, T1, NRANGES[0], n0_pad)
    plan2 = _plan_gather(tiles2, T2, NRANGES[1], n1_pad)
    _fill_gather(plan1, tiles1)
    _fill_gather(plan2, tiles2)
    assert n0_pad <= WINDOW and n1_pad <= WINDOW

    return dict(
        plan0=plan0,
        plans=(plan1, plan2),
        T=(T0, T1, T2),
        n0_pad=n0_pad,
        n1_pad=n1_pad,
        xsrc8s=xsrc8s,
        xdstTs=xdstTs,
        blocks=blocks,
        weights=tuple(
            (
                np.asarray(inputs[f"W_self{l}"], np.float32),
                np.asarray(inputs[f"W_neigh{l}"], np.float32),
                np.asarray(inputs[f"b{l}"], np.float32),
            )
            for l in range(3)
        ),
    )


# ---------------------------------------------------------------------------
# Device kernel
# ---------------------------------------------------------------------------


def _wrap_idx16(plan, c):
    """int16 idx table: flat stream wrapped into 16 partitions, replicated
    to 128 (the gather ucode reads 16-partition-wrapped indices)."""
    flat = plan.gidx[c].T.reshape(-1)  # stream order
    w = flat.reshape(len(flat) // 16, 16).T.astype(np.int16)
    out = np.zeros((P, w.shape[1]), np.int16)
    out[:16] = w
    for rep in range(1, 8):
        out[rep * 16 : (rep + 1) * 16] = out[:16]
    return out


def run_device(meta, trace=False):
    import concourse.bacc as bacc
    import concourse.tile as tile
    import concourse.mybir as mybir
    from concourse.bass_utils import run_bass_kernel_spmd

    plan0 = meta["plan0"]
    plans = meta["plans"]
    f32 = mybir.dt.float32
    b16 = mybir.dt.bfloat16
    f8e4 = mybir.dt.float8e4
    DR = mybir.MatmulPerfMode.DoubleRow

    nc = bacc.Bacc("TRN2", target_bir_lowering=False, debug=False, num_devices=NCORES)

    xsrc8 = nc.dram_tensor("xsrc8", [P, plan0.n_sp_cols * FEAT], f8e4,
                           kind="ExternalInput")
    xdstT = nc.dram_tensor("xdstT", [P, plan0.n_tiles * FEAT], b16,
                           kind="ExternalInput")
    sp0_d = nc.dram_tensor("sp0", [P, plan0.n_sp_cols * P], f8e4,
                           kind="ExternalInput")
    ident_d = nc.dram_tensor("ident", [P, P], b16, kind="ExternalInput")
    ones_d = nc.dram_tensor("ones", [1, P], b16, kind="ExternalInput")
    h1buf = nc.dram_tensor("h1buf", [meta["n0_pad"], FEAT], b16)
    h2buf = nc.dram_tensor("h2buf", [meta["n1_pad"], FEAT], b16)
    out_d = nc.dram_tensor("out", [SEEDS_PER_CORE, OUTW[2]], f32, kind="ExternalOutput")

    idx_d, sp_d = [], []
    for li, plan in enumerate(plans):
        idx_d.append(
            nc.dram_tensor(f"gidx{li + 1}", [P, plan.n_chunks * P // 16],
                           mybir.dt.int16, kind="ExternalInput")
        )
        sp_d.append(
            nc.dram_tensor(f"sp{li + 1}", [P, plan.n_sp_cols * P], b16,
                           kind="ExternalInput")
        )
    w_d = []
    for l in range(3):
        w_d.append(
            (
                nc.dram_tensor(f"ws{l}", [FEAT, OUTW[l]], b16, kind="ExternalInput"),
                nc.dram_tensor(f"wn{l}", [FEAT, OUTW[l]], b16, kind="ExternalInput"),
                nc.dram_tensor(f"bias{l}", [P, OUTW[l]], b16, kind="ExternalInput"),
            )
        )

    Kmax = max(plan0.K)

    with tile.TileContext(nc) as tc:
        with (
            tc.tile_pool(name="const", bufs=1) as cpool,
            tc.tile_pool(name="msgs", bufs=4) as mpool,
            tc.tile_pool(name="sel", bufs=4) as spool,
            tc.tile_pool(name="gat", bufs=1) as gpool,
            tc.tile_pool(name="acc", bufs=3) as apool,
            tc.tile_pool(name="outp", bufs=3) as opool,
            tc.tile_pool(name="pagg", bufs=2, space="PSUM") as pa,
            tc.tile_pool(name="py", bufs=2, space="PSUM") as pypool,
        ):
            ident_t = cpool.tile([P, P], b16, tag="ident")
            nc.sync.dma_start(out=ident_t[:], in_=ident_d[:])
            ws_ts, wn_ts, bias_ts = [], [], []
            for l in range(3):
                outw = OUTW[l]
                wst, wnt = [], []
                for k in range(2):
                    w = cpool.tile([P, outw], b16, tag=f"ws{l}_{k}")
                    nc.sync.dma_start(out=w[:], in_=w_d[l][0][k * P : (k + 1) * P, :])
                    wst.append(w)
                    w = cpool.tile([P, outw], b16, tag=f"wn{l}_{k}")
                    nc.sync.dma_start(out=w[:], in_=w_d[l][1][k * P : (k + 1) * P, :])
                    wnt.append(w)
                ws_ts.append(wst)
                wn_ts.append(wnt)
                # bias broadcast across partitions: PSUM gets pre-filled with
                # it by the DVE, replacing a per-tile PE bias matmul
                bias_t = cpool.tile([P, outw], b16, tag=f"bias{l}")
                nc.sync.dma_start(out=bias_t[:], in_=w_d[l][2][:])
                bias_ts.append(bias_t)
            idx_ts = []
            for li, plan in enumerate(plans):
                idx_t = cpool.tile(
                    list(idx_d[li].shape), mybir.dt.int16, tag=f"idx{li + 1}"
                )
                nc.sync.dma_start(out=idx_t[:], in_=idx_d[li][:])
                idx_ts.append(idx_t)

            def agg_to_acT(pag):
                """agg [128d, 256f] psum -> bf16 -> PE-transpose -> acT
                [128f-half x2, 128d] sbuf (feature-major for the tail)."""
                ag = apool.tile([P, FEAT], b16, tag="ag")
                nc.vector.tensor_copy(out=ag[:], in_=pag[:])
                ptr = pa.tile([P, FEAT], b16, tag="ptr")
                for h in range(2):
                    nc.tensor.matmul(ptr[:, h * P : (h + 1) * P],
                                     lhsT=ag[:, h * P : (h + 1) * P],
                                     rhs=ident_t[:], is_transpose=True)
                acT = apool.tile([P, FEAT], b16, tag="acT")
                nc.vector.tensor_copy(out=acT[:], in_=ptr[:])
                return acT

            # (L1/L2 psum tags alias L0's pcA slot size: [P, 256] f32)

            biases_nonzero = [
                bool(np.any(meta["weights"][l][2] != 0.0)) for l in range(3)
            ]

            def tile_tail(l, t, a0, a1, hd0, hd1, dest):
                """Y matmuls + activation + store for one dst tile.
                a0/a1: aggT feature halves [128f, 128d] (bf16);
                hd0/hd1: h_dstT feature halves [128f, 128d] (bf16).
                A nonzero bias is added by the DVE on the way out."""
                outw = OUTW[l]
                y = pypool.tile([P, outw], f32, tag="y")
                nc.tensor.matmul(y[:], lhsT=a0, rhs=wn_ts[l][0][:],
                                 start=True, stop=False)
                nc.tensor.matmul(y[:], lhsT=a1, rhs=wn_ts[l][1][:],
                                 start=False, stop=False)
                nc.tensor.matmul(y[:], lhsT=hd0, rhs=ws_ts[l][0][:],
                                 start=False, stop=False)
                nc.tensor.matmul(y[:], lhsT=hd1, rhs=ws_ts[l][1][:],
                                 start=False, stop=True)
                yv = y[:]
                if biases_nonzero[l]:
                    yb = apool.tile([P, outw], f32, tag="yb")
                    nc.vector.tensor_tensor(
                        out=yb[:], in0=y[:], in1=bias_ts[l][:],
                        op=mybir.AluOpType.add,
                    )
                    yv = yb[:]
                if l < 2:
                    o2 = opool.tile([P, outw], b16, tag="o2")
                    nc.vector.tensor_scalar_max(out=o2[:], in0=yv, scalar1=0.0)
                    nc.sync.dma_start(out=dest[t * P : (t + 1) * P, :], in_=o2[:])
                else:
                    o = opool.tile([P, outw], f32, tag="o")
                    nc.vector.tensor_copy(out=o[:], in_=yv)
                    nc.sync.dma_start(out=dest[:], in_=o[0:SEEDS_PER_CORE, :])

            # ================= layer 0: fp8 bands, DoubleRow agg =============
            pend = None
            for t in range(plan0.n_tiles):
                K = plan0.K[t]
                so = int(plan0.sp_off[t])
                bt = mpool.tile([P, Kmax * FEAT], f8e4, tag="band")
                nc.scalar.dma_start(
                    out=bt[:, : K * FEAT],
                    in_=xsrc8[:, so * FEAT : (so + K) * FEAT],
                )
                spt = spool.tile([P, Kmax * P], f8e4, tag="spb")
                nc.scalar.dma_start(
                    out=spt[:, : K * P], in_=sp0_d[:, so * P : (so + K) * P]
                )
                hdt = apool.tile([P, FEAT], b16, tag="hdt")
                nc.sync.dma_start(
                    out=hdt[:], in_=xdstT[:, t * FEAT : (t + 1) * FEAT]
                )
                # the two feature-half accumulation groups run SEQUENTIALLY:
                # start=True lazily zero-arms the whole 2KB PSUM region, so
                # interleaving groups in one bank corrupts the sibling's
                # partial sums
                pcA = pa.tile([P, 2 * P], f32, tag="pcA")
                npair = K // 2
                odd = K % 2
                for h in range(2):
                    dst = pcA[:, h * P : (h + 1) * P]
                    for kp in range(npair):
                        st, sp = (kp == 0), (kp == npair - 1 and not odd)
                        bv = bt[:, 2 * kp * FEAT : (2 * kp + 2) * FEAT].rearrange(
                            "p (k f) -> p k f", k=2
                        )
                        sv = spt[:, 2 * kp * P : (2 * kp + 2) * P].rearrange(
                            "p (k d) -> p k d", k=2
                        )
                        nc.tensor.matmul(dst, lhsT=bv[:, :, h * P : (h + 1) * P],
                                         rhs=sv[:], start=st, stop=sp,
                                         perf_mode=DR)
                    if odd:
                        k = K - 1
                        base = k * FEAT + h * P
                        nc.tensor.matmul(dst, lhsT=bt[:, base : base + P],
                                         rhs=spt[:, k * P : (k + 1) * P],
                                         start=(npair == 0), stop=True)
                ac = apool.tile([P, 2 * P], b16, tag="ac")
                nc.vector.tensor_copy(out=ac[:], in_=pcA[:])
                # software-pipeline: emit the PREVIOUS tile's tail now, so
                # its DVE psum->sbuf copy overlapped this tile's agg matmuls
                # and the tensor queue never stalls on the copy
                if pend is not None:
                    tile_tail(0, *pend)
                pend = (t, ac[:, 0:P], ac[:, P : 2 * P],
                        hdt[:, 0:P], hdt[:, P : 2 * P], h1buf)

            if pend is not None:
                tile_tail(0, *pend)

            # ============ layers 1/2: range-split gather + bf16 agg ==========
            tables = [h1buf, h2buf]
            dests = [h2buf, out_d]
            for li, plan in enumerate(plans):
                l = li + 1
                table, dest = tables[li], dests[li]
                idx_t = idx_ts[li]
                nch = plan.n_chunks
                nrows = table.shape[0]

                mt = gpool.tile([P, nch * FEAT], b16, tag=f"msgs{l}")
                for r in range(plan.n_ranges):
                    a = plan.range_chunk_off[r]
                    b2 = plan.range_chunk_off[r + 1]
                    if b2 == a:
                        continue
                    hi = min((r + 1) * plan.step_rows, nrows)
                    nc.gpsimd.dma_gather(
                        out_ap=mt[:, a * FEAT : b2 * FEAT].rearrange(
                            "p (g d) -> p g d", g=b2 - a
                        ),
                        in_ap=table[0:hi, :],
                        idxs_ap=idx_t[:, a * P // 16 : b2 * P // 16],
                        num_idxs=(b2 - a) * P,
                        num_idxs_reg=(b2 - a) * P,
                        elem_size=FEAT,
                        single_packet=False,
                    )

                SPG = 16
                n_slabs = -(-plan.n_sp_cols // SPG)
                sp_tiles = []
                for k in range(n_slabs):
                    c0 = k * SPG * P
                    c1 = min((k + 1) * SPG * P, plan.n_sp_cols * P)
                    st = spool.tile([P, SPG * P], b16, tag=f"sp{l}_{k}", bufs=1)
                    nc.scalar.dma_start(out=st[:, : c1 - c0], in_=sp_d[li][:, c0:c1])
                    sp_tiles.append(st)

                def sp_slice(col):
                    k, j = divmod(col, SPG)
                    return sp_tiles[k][:, j * P : (j + 1) * P]

                pend = None
                for t in range(plan.n_tiles):
                    hd = opool.tile([P, FEAT], b16, tag="hd")
                    nc.scalar.dma_start(out=hd[:], in_=table[t * P : (t + 1) * P, :])
                    pcd = pa.tile([P, 2 * P], f32, tag="pcd")
                    nc.tensor.matmul(pcd[:, 0:P], lhsT=hd[:, 0:P],
                                     rhs=ident_t[:], start=True, stop=True)
                    nc.tensor.matmul(pcd[:, P : 2 * P], lhsT=hd[:, P : 2 * P],
                                     rhs=ident_t[:], start=True, stop=True)
                    pag = pa.tile([P, FEAT], f32, tag="pcA")
                    pairs = plan.tile_pairs[t]
                    for i, (sp_col, ch) in enumerate(pairs):
                        st, sp = (i == 0), (i == len(pairs) - 1)
                        nc.tensor.matmul(pag[:], lhsT=sp_slice(sp_col),
                                         rhs=mt[:, ch * FEAT : (ch + 1) * FEAT],
                                         start=st, stop=sp)
                    hdT = apool.tile([P, 2 * P], b16, tag="hdT")
                    nc.vector.tensor_copy(out=hdT[:], in_=pcd[:])
                    if pend is not None:
                        acT = agg_to_acT(pend[0])
                        tile_tail(l, pend[1], acT[:, 0:P], acT[:, P : 2 * P],
                                  pend[2][:, 0:P], pend[2][:, P : 2 * P], dest)
                    pend = (pag, t, hdT)
                if pend is not None:
                    acT = agg_to_acT(pend[0])
                    tile_tail(l, pend[1], acT[:, 0:P], acT[:, P : 2 * P],
                              pend[2][:, 0:P], pend[2][:, P : 2 * P], dest)

    nc.compile()

    in_maps = []
    bf16 = _bf16()
    eye16 = np.eye(P, dtype=bf16)
    for c in range(NCORES):
        m = dict(
            xsrc8=meta["xsrc8s"][c],
            xdstT=meta["xdstTs"][c],
            sp0=np.ascontiguousarray(
                _q8(plan0.wmat[c]).reshape(P, plan0.n_sp_cols * P)
            ),
            ident=eye16,
            ones=np.ones((1, P), dtype=bf16),
        )
        for li, plan in enumerate(plans):
            m[f"gidx{li + 1}"] = _wrap_idx16(plan, c)
            m[f"sp{li + 1}"] = np.ascontiguousarray(
                plan.wmat[c].astype(bf16).reshape(P, plan.n_sp_cols * P)
            )
        for l in range(3):
            ws, wn, b = meta["weights"][l]
            m[f"ws{l}"] = np.ascontiguousarray(ws.astype(bf16))
            m[f"wn{l}"] = np.ascontiguousarray(wn.astype(bf16))
            m[f"bias{l}"] = np.ascontiguousarray(
                np.tile(b[None, :], (P, 1)).astype(bf16)
            )
        in_maps.append(m)

    res = run_bass_kernel_spmd(
        nc, in_maps, core_ids=list(range(NCORES)), trace=trace
    )
    return [res.results[c]["out"] for c in range(NCORES)], res


def assemble(meta, outs):
    full = np.zeros((NUM_DST[2], OUTW[2]), np.float32)
    for c in range(NCORES):
        full[meta["blocks"][c]["seeds"]] = outs[c]
    return full


def kernel(**inputs) -> np.ndarray:
    meta = build_host(inputs)
    outs, _ = run_device(meta)
    return assemble(meta, outs)
